# revision 9
# baseline (speedup 1.0000x reference)
"""Trainium2 Bass kernel: 10-layer LSTM (D=25) + FC(7) + softmax.

Strategy v2: hybrid time x batch sharding. 8 cores = 4 time-shards x 2
batch-shards (BS=256 per core). Each time-shard runs T_local = 512 + 32
warmup steps (warmup re-converges the recurrent state from zeros; decay
~0.5/step makes 32 steps ~1e-7 accurate); shard 0 starts from the true
(h0, c0) so needs no warmup but runs the same uniform program.

Inside each core: anti-diagonal wavefront over (layer, time), layers
grouped into quads q0 = layers 0-3, q1 = 4-7, q2 = 8-9. Per quad and
window, FOUR gate-major matmuls (one per gate i/f/g/o, M = dense 25-row
cell slots) read one K-stacked "H-tile" [recurrent h-slots | input slot |
ones row] so every elementwise op downstream is a dense full-height op
with identical row layout (no partition-alignment juggling).

All activations are SIGMOID ONLY: tanh(x) = 2*sigmoid(2x) - 1 is folded
algebraically - g-gate weights carry an extra x2, the cell state is
tracked as c2 = 2c, and h is tracked as h~ = h/2 (the x2 compensations
live in the packed weights). Cell math per quad is then 4 DVE ops:
  t1 = (sg - 0.5) * si          [scalar_tensor_tensor]
  v  = sf * c2_old              [tensor_tensor]
  c2 = (t1 * 4) + v             [scalar_tensor_tensor]
  h~ = (sc - 0.5) * so          [scalar_tensor_tensor]
Everything on-device is fp16 (validated: softmax rel err ~2e-4).

FC head: 4-band column packing (chunk 4j+g in psum rows 32g:32g+7),
exp + per-band sums on device; final softmax divide + unpack on host.
"""
import sys, os

for _p in ("/opt/trn_rl_repo", "/root/.axon_site/_ro/trn_rl_repo"):
    if os.path.isdir(_p) and _p not in sys.path:
        sys.path.insert(0, _p)

# The agent image's `antenv` lacks `axon_hooks`, which bass_utils imports
# unconditionally when trace=True. Shim it (and register the NTFF hook so
# HW profiling works) before importing bass_utils.
try:
    import antenv.axon_hooks  # noqa: F401
except ImportError:
    import types
    import antenv

    _HOOK = [None]
    _m = types.ModuleType("antenv.axon_hooks")
    _m.get_axon_ntff_profile_hook = lambda: _HOOK[0]
    _m.set_axon_ntff_profile_hook = lambda h: _HOOK.__setitem__(0, h)
    sys.modules["antenv.axon_hooks"] = _m
    antenv.axon_hooks = _m
    try:
        from trn_agent_boot.trn_boot import _ntff_profile_via_ctypes

        _m.set_axon_ntff_profile_hook(
            _ntff_profile_via_ctypes("/opt/axon/libaxon_pjrt.so"))
    except Exception:
        pass

import numpy as np
import concourse.bass as bass
import concourse.mybir as mybir
from concourse.tile import TileContext
from concourse.bass_utils import run_bass_kernel_spmd

F32 = mybir.dt.float32
F16 = mybir.dt.float16
AF = mybir.ActivationFunctionType
ALU = mybir.AluOpType

T, B, D, L, NCLS = 2048, 512, 25, 10, 7
NC = 8
NT, NBS = 4, 2            # time shards x batch shards
BS = B // NBS             # 256
WARM = 32
QUADS = [(0, 4), (4, 4), (8, 2)]   # (first layer, n cells)
GATE_SL = {"i": slice(0, 25), "f": slice(25, 50), "g": slice(50, 75),
           "o": slice(75, 100)}


# ---------------------------------------------------------------- weights
def _pack_weights(W_ih, W_hh, b, fc_w, fc_b):
    """One SBUF blob [128, ncols] fp16.

    Quad gate mats (12): [Kq, 128]; K rows: slot j = cell j's recurrent
    h~ (rows 25j:25j+25), which also feeds cell j+1's input weights;
    input slot (rows 25nl:25nl+25) = cell 0's input; ones row = biases.
    x2 on all weight cols consuming h~ inputs; another x2 on g-gate.
    FC mats (4): [26, 103] band g at rows 32g:32g+7 (fc_w x2: h~ input).
    Selector [103, 4].
    """
    mats, cols = [], {}

    def add(key, m):
        cols[key] = sum(x.shape[1] for x in mats)
        mats.append(m)

    for q, (l0, nl) in enumerate(QUADS):
        Kq = 25 * nl + 26
        for gate in "ifgo":
            gs = GATE_SL[gate]
            sg = 2.0 if gate == "g" else 1.0
            m = np.zeros((Kq, 128), np.float32)
            for j in range(nl):
                l = l0 + j
                m[25 * j:25 * j + 25, 25 * j:25 * j + 25] = \
                    (2.0 * sg * W_hh[l][gs]).T
                if j >= 1:
                    m[25 * (j - 1):25 * j, 25 * j:25 * j + 25] = \
                        (2.0 * sg * W_ih[l][gs]).T
                m[Kq - 1, 25 * j:25 * j + 25] = sg * b[l][gs]
            sx = 1.0 if q == 0 else 2.0
            m[25 * nl:25 * nl + 25, 0:25] = (sx * sg * W_ih[l0][gs]).T
            add((q, gate), m)

    for g in range(4):
        m = np.zeros((26, 103), np.float32)
        m[0:25, 32 * g:32 * g + 7] = (2.0 * fc_w).T
        m[25, 32 * g:32 * g + 7] = fc_b
        add(("fc", g), m)
    m = np.zeros((103, 4), np.float32)
    for g in range(4):
        m[32 * g:32 * g + 7, g] = 1.0
    add(("sel",), m)

    ncols = sum(x.shape[1] for x in mats)
    blob = np.zeros((128, ncols), np.float16)
    c = 0
    for m in mats:
        blob[:m.shape[0], c:c + m.shape[1]] = m.astype(np.float16)
        c += m.shape[1]
    return blob, cols


_WFCTR = [0]


def _split_excess_waits(nc):
    """Walrus allows 1 sync-wait per instruction (2 for EventSemaphore).
    Hoist extras onto preceding same-engine NOPs."""
    for fn in nc.m.functions:
        for blk in fn.blocks:
            insts = list(blk.instructions)
            out = []
            changed = False
            for inst in insts:
                si = inst.sync_info
                cap = 2 if type(inst).__name__ == "InstEventSemaphore" else 1
                if si is not None and si.on_wait is not None and len(si.on_wait) > cap:
                    waits = list(si.on_wait)
                    extra, keep = waits[:-cap], waits[-cap:]
                    for wt in extra:
                        _WFCTR[0] += 1
                        out.append(mybir.InstNoOp(
                            name=f"I-waitfix-{_WFCTR[0]}", opcode="NoOp",
                            engine=inst.engine, ins=[], outs=[],
                            sync_info=mybir.SyncInfo(on_wait=[wt], on_update=[]),
                        ))
                    inst.sync_info = mybir.SyncInfo(
                        on_wait=keep, on_update=list(si.on_update))
                    changed = True
                out.append(inst)
            if changed:
                blk.instructions = out


# ---------------------------------------------------------------- program
_PROG_CACHE = {}


def _build_program(TL, wcols, nwcol, patch_waits=True):
    nc = bass.Bass()
    NW = TL + L - 1
    NCH = TL * BS // 512
    NGRP = NCH // 4

    xT = nc.declare_dram_parameter("xT", [25, (TL + 16) * BS], F16, isOutput=False)
    wpack = nc.declare_dram_parameter("wpack", [128, nwcol], F16, isOutput=False)
    hpre = nc.declare_dram_parameter("hpre", [100, 3 * BS], F16, isOutput=False)
    cpre = nc.declare_dram_parameter("cpre", [100, 3 * BS], F16, isOutput=False)
    onesd = nc.declare_dram_parameter("onesd", [1, 512], F16, isOutput=False)
    exp_out = nc.declare_dram_parameter("exp_out", [NGRP, 4, 7, 512], F16, isOutput=True)
    sum_out = nc.declare_dram_parameter("sum_out", [NGRP, 4, 512], F32, isOutput=True)
    h9d = nc.dram_tensor("h9d", [25, TL * BS], F16)

    with TileContext(nc) as tc:
      with tc.tile_pool(name="glob", bufs=1) as glob:
        wsb = glob.tile([128, nwcol], F16)
        with (
            tc.tile_pool(name="pers", bufs=1) as pers,
            tc.tile_pool(name="wps", bufs=1, space="PSUM") as pps,
        ):
            Ht = [[pers.tile([128, BS], F16, name=f"H{q}p{p}") for p in range(2)]
                  for q in range(3)]
            Ct = [pers.tile([128, 3 * BS], F16, name=f"C{p}") for p in range(2)]
            SIF = [[pers.tile([128, 2 * BS], F16, name=f"SIF{q}p{p}") for p in range(2)]
                   for q in range(3)]
            SGO = [[pers.tile([128, 2 * BS], F16, name=f"SGO{q}p{p}") for p in range(2)]
                   for q in range(3)]
            SC = [pers.tile([128, 3 * BS], F16, name=f"SC{p}") for p in range(2)]
            T1 = [pers.tile([128, BS], F16, name=f"T1{q}") for q in range(3)]
            Vt = [pers.tile([128, BS], F16, name=f"V{q}") for q in range(3)]
            pIF = [pps.tile([128, 2 * BS], F32, name=f"pIF{q}") for q in range(3)]
            pGO = [pps.tile([128, 2 * BS], F32, name=f"pGO{q}") for q in range(3)]

            # ---- init
            nc.sync.dma_start(out=wsb[:, :], in_=wpack[:, :])
            for q, (l0, nl) in enumerate(QUADS):
                R = 25 * nl
                for p in range(2):
                    nc.sync.dma_start(
                        out=Ht[q][p][0:R, :],
                        in_=hpre[0:R, q * BS:(q + 1) * BS])
                    nc.sync.dma_start(
                        out=Ht[q][p][R + 25:R + 26, :], in_=onesd[:, 0:BS])
            for p in range(2):
                nc.sync.dma_start(out=Ct[p][0:100, :], in_=cpre[:, :])
            # x(0) -> H0 parity1 input slot, x(1) -> parity0
            nc.sync.dma_start(out=Ht[0][1][100:125, :], in_=xT[:, 0:BS])
            nc.sync.dma_start(out=Ht[0][0][100:125, :], in_=xT[:, BS:2 * BS])

            # ---- wavefront
            for w in range(NW):
                p, pp = w % 2, (w + 1) % 2
                act = [q for q, (l0, nl) in enumerate(QUADS) if w >= l0]
                for q in act:
                    l0, nl = QUADS[q]
                    R = 25 * nl
                    Kq = R + 26
                    for gate in "ifgo":
                        bank = pIF[q] if gate in "if" else pGO[q]
                        co = 0 if gate in "ig" else BS
                        wc = wcols[(q, gate)]
                        nc.tensor.matmul(
                            bank[0:128, co:co + BS],
                            wsb[0:Kq, wc:wc + 128],
                            Ht[q][pp][0:Kq, :],
                            start=True, stop=True,
                        )
                    nc.scalar.activation(
                        SGO[q][p][0:R, :], pGO[q][0:R, :], AF.Sigmoid)
                    nc.scalar.activation(
                        SIF[q][p][0:R, :], pIF[q][0:R, :], AF.Sigmoid)
                    # t1 = (sg - 0.5) * si ; v = sf * c2_old
                    nc.vector.scalar_tensor_tensor(
                        T1[q][0:R, :], SGO[q][p][0:R, 0:BS], -0.5,
                        SIF[q][p][0:R, 0:BS], ALU.add, ALU.mult)
                    nc.vector.tensor_mul(
                        Vt[q][0:R, :], SIF[q][p][0:R, BS:2 * BS],
                        Ct[pp][0:R, q * BS:q * BS + BS])
                    rw = min(R, 25 * (w - l0 + 1))
                    nc.vector.scalar_tensor_tensor(
                        Ct[p][0:rw, q * BS:q * BS + BS], T1[q][0:rw, :], 4.0,
                        Vt[q][0:rw, :], ALU.mult, ALU.add)
                # sigma(c2') for all quads at once
                nc.scalar.activation(
                    SC[p][0:100, :], Ct[p][0:100, :], AF.Sigmoid)
                for q in act:
                    l0, nl = QUADS[q]
                    rw = min(25 * nl, 25 * (w - l0 + 1))
                    nc.vector.scalar_tensor_tensor(
                        Ht[q][p][0:rw, :], SC[p][0:rw, q * BS:q * BS + BS],
                        -0.5, SGO[q][p][0:rw, BS:2 * BS], ALU.add, ALU.mult)
                # input-slot feeds for next window
                if w >= 3:
                    nc.gpsimd.dma_start(
                        out=Ht[1][p][100:125, :], in_=Ht[0][p][75:100, :])
                if w >= 7:
                    nc.gpsimd.dma_start(
                        out=Ht[2][p][50:75, :], in_=Ht[1][p][75:100, :])
                # x prefetch for window w+2 (reads H0[pp] next window: WAR ok)
                nc.sync.dma_start(
                    out=Ht[0][pp][100:125, :],
                    in_=xT[:, (w + 2) * BS:(w + 3) * BS])
                # h9 export
                if w >= 9:
                    nc.sync.dma_start(
                        out=h9d[:, (w - 9) * BS:(w - 8) * BS],
                        in_=Ht[2][p][25:50, :])

        # ---------------- tail: FC + exp + band sums
        with (
            tc.tile_pool(name="tailw", bufs=4) as twp,
            tc.tile_pool(name="tps", bufs=2, space="PSUM") as tps,
            tc.tile_pool(name="tpers", bufs=1) as tpers,
        ):
            rhs = [tpers.tile([26, 512], F16, name=f"rhs{i}") for i in range(2)]
            for i in range(2):
                nc.sync.dma_start(out=rhs[i][25:26, :], in_=onesd[:, :])
            for j in range(NGRP):
                fcps = tps.tile([128, 512], F32, tag="fcps")
                for g in range(4):
                    ch = 4 * j + g
                    rt = rhs[ch % 2]
                    nc.sync.dma_start(
                        out=rt[0:25, :], in_=h9d[:, ch * 512:(ch + 1) * 512])
                    nc.tensor.matmul(
                        fcps[0:103, :],
                        wsb[0:26, wcols[("fc", g)]:wcols[("fc", g)] + 103],
                        rt[0:26, :],
                        start=(g == 0), stop=(g == 3),
                    )
                esb = twp.tile([128, 512], F16, tag="esb")
                nc.scalar.activation(esb[0:103, :], fcps[0:103, :], AF.Exp)
                sps = tps.tile([4, 512], F32, tag="sps")
                nc.tensor.matmul(
                    sps[0:4, :],
                    wsb[0:103, wcols[("sel",)]:wcols[("sel",)] + 4],
                    esb[0:103, :],
                    start=True, stop=True,
                )
                ssb = twp.tile([32, 512], F32, tag="ssb")
                nc.scalar.copy(ssb[0:4, :], sps[0:4, :])
                for g in range(4):
                    nc.gpsimd.dma_start(
                        out=exp_out[j, g, :, :], in_=esb[32 * g:32 * g + 7, :])
                nc.sync.dma_start(out=sum_out[j, :, :], in_=ssb[0:4, :])

    if patch_waits:
        _split_excess_waits(nc)
    return nc


def _get_program(TL, wcols, nwcol):
    if TL not in _PROG_CACHE:
        _PROG_CACHE[TL] = _build_program(TL, wcols, nwcol)
    return _PROG_CACHE[TL]


# ---------------------------------------------------------------- kernel
def kernel(x, h0, c0, W_ih, W_hh, b, fc_w, fc_b, _trace=False, _TL=None):
    x = np.asarray(x, np.float32)
    h0 = np.asarray(h0, np.float32)
    c0 = np.asarray(c0, np.float32)
    TL = (T // NT + WARM) if _TL is None else _TL
    TCH = TL - WARM                 # real timesteps per time-shard
    Teff = NT * TCH

    blob, wcols = _pack_weights(
        np.asarray(W_ih, np.float32), np.asarray(W_hh, np.float32),
        np.asarray(b, np.float32), np.asarray(fc_w, np.float32),
        np.asarray(fc_b, np.float32))

    nc = _get_program(TL, wcols, blob.shape[1])

    in_maps = []
    for tb in range(NT):
        t0 = 0 if tb == 0 else tb * TCH - WARM
        for bb in range(NBS):
            bsl = slice(bb * BS, (bb + 1) * BS)
            xt = np.zeros((25, (TL + 16) * BS), np.float16)
            xc = x[t0:t0 + TL, bsl, :]              # [TL, BS, 25]
            xt[:, 0:TL * BS] = xc.transpose(2, 0, 1).reshape(25, -1)
            hp = np.zeros((100, 3 * BS), np.float16)
            cp = np.zeros((100, 3 * BS), np.float16)
            if tb == 0:
                for q, (l0, nl) in enumerate(QUADS):
                    for j in range(nl):
                        hp[25 * j:25 * j + 25, q * BS:(q + 1) * BS] = \
                            (h0[l0 + j, bsl, :].T / 2).astype(np.float16)
                        cp[25 * j:25 * j + 25, q * BS:(q + 1) * BS] = \
                            (2 * c0[l0 + j, bsl, :].T).astype(np.float16)
            in_maps.append({"xT": xt, "wpack": blob, "hpre": hp, "cpre": cp,
                            "onesd": np.ones((1, 512), np.float16)})

    res = run_bass_kernel_spmd(nc, in_maps, list(range(NC)), trace=_trace)

    # host: softmax divide + assemble
    y = np.empty((Teff, B, NCLS), np.float32)
    ci = 0
    for tb in range(NT):
        for bb in range(NBS):
            e = res.results[ci]["exp_out"].astype(np.float32)  # [NGRP,4,7,512]
            s = res.results[ci]["sum_out"]                     # [NGRP,4,512]
            yc = (e / s[:, :, None, :]).transpose(0, 1, 3, 2)  # [NGRP,4,512,7]
            yc = yc.reshape(TL, BS, NCLS)
            tsl = yc[0:TCH] if tb == 0 else yc[WARM:WARM + TCH]
            y[tb * TCH:(tb + 1) * TCH, bb * BS:(bb + 1) * BS, :] = tsl
            ci += 1
    out = y.reshape(Teff * B, NCLS)
    return (out, res) if _trace else out


if __name__ == "__main__":
    pass


# revision 10
# speedup vs baseline: 1.6251x; 1.6251x over previous
"""Trainium2 Bass kernel: 10-layer LSTM (D=25) + FC(7) + softmax.

Strategy v3: hybrid time x batch sharding. 8 cores = 4 time-shards x 2
batch-shards (BS=256 per core). Each time-shard runs T_local = 512 + 32
warmup steps (warmup re-converges the recurrent state from zeros; decay
~0.5/step makes 32 steps ~1e-7 accurate); shard 0 starts from the true
(h0, c0) and the uniform program just computes 32 extra tail steps.

Inside each core: anti-diagonal wavefront over (layer, time); layers in
quads q0 = 0-3, q1 = 4-7, q2 = 8-9, with quad q additionally skewed by
+q windows so the cross-quad h hand-off (an SBUF->SBUF DMA into the next
quad's K-stack input slot) gets a full 2-window slack and stays off the
recurrent critical path.

Per quad and window: FOUR gate-major matmuls (M = dense 25-row cell
slots, K = the quad's H-tile [recurrent h slots | input slot | ones])
into one 2-bank PSUM tile [I|F|G|O]; ONE sigmoid over all four gate
blocks (g-gate weights carry x2 so tanh(g) = 2*sig(2g)-1 is recovered on
the DVE); then dense same-row DVE ops:
  w  = sg * 2 - 1         [tensor_scalar, 4x]
  u  = w * si             [tensor_tensor, 2x]
  v  = sf * c_old         [tensor_tensor, 2x]
  c' = u + v              [tensor_tensor, 2x]
  z  = tanh(c')           [scalar ACT, same table as sigmoid]
  h  = z * so             [tensor_tensor, 2x]
Everything on-device is fp16 (validated ~5e-4).

FC head: 4-band column packing (chunk 4j+g in psum rows 32g:32g+7),
exp + per-band sums on device; final softmax divide + unpack on host.
"""
import sys, os

for _p in ("/opt/trn_rl_repo", "/root/.axon_site/_ro/trn_rl_repo"):
    if os.path.isdir(_p) and _p not in sys.path:
        sys.path.insert(0, _p)

# The agent image's `antenv` lacks `axon_hooks`, which bass_utils imports
# unconditionally when trace=True. Shim it (and register the NTFF hook so
# HW profiling works) before importing bass_utils.
try:
    import antenv.axon_hooks  # noqa: F401
except ImportError:
    import types
    import antenv

    _HOOK = [None]
    _m = types.ModuleType("antenv.axon_hooks")
    _m.get_axon_ntff_profile_hook = lambda: _HOOK[0]
    _m.set_axon_ntff_profile_hook = lambda h: _HOOK.__setitem__(0, h)
    sys.modules["antenv.axon_hooks"] = _m
    antenv.axon_hooks = _m
    try:
        from trn_agent_boot.trn_boot import _ntff_profile_via_ctypes

        _m.set_axon_ntff_profile_hook(
            _ntff_profile_via_ctypes("/opt/axon/libaxon_pjrt.so"))
    except Exception:
        pass

import numpy as np
import concourse.bass as bass
import concourse.mybir as mybir
from concourse.tile import TileContext
from concourse.bass_utils import run_bass_kernel_spmd

F32 = mybir.dt.float32
F16 = mybir.dt.float16
AF = mybir.ActivationFunctionType
ALU = mybir.AluOpType

T, B, D, L, NCLS = 2048, 512, 25, 10, 7
NC = 8
NT, NBS = 4, 2            # time shards x batch shards
BS = B // NBS             # 256
WARM = 32
QUADS = [(0, 4), (4, 4), (8, 2)]   # (first layer, n cells)
SKEW = [0, 1, 2]                   # extra window lag per quad
GATE_SL = {"i": slice(0, 25), "f": slice(25, 50), "g": slice(50, 75),
           "o": slice(75, 100)}
PIPE = 11                          # wavefront depth: layer 9 lands at w = t+11


# ---------------------------------------------------------------- weights
def _pack_weights(W_ih, W_hh, b, fc_w, fc_b):
    """One SBUF blob [128, ncols] fp16.

    Quad gate mats (12): [Kq, 128]; K rows: slot j = cell j's recurrent
    h (rows 25j:25j+25), which also feeds cell j+1's input weights;
    input slot (rows 25nl:25nl+25) = cell 0's input; ones row = biases.
    g-gate carries x2 (tanh(g) = 2 sigmoid(2g) - 1 recovered on DVE).
    FC mats (4): [26, 103] band g at rows 32g:32g+7. Selector [103, 4].
    """
    mats, cols = [], {}

    def add(key, m):
        cols[key] = sum(x.shape[1] for x in mats)
        mats.append(m)

    for q, (l0, nl) in enumerate(QUADS):
        Kq = 25 * nl + 26
        for gate in "ifgo":
            gs = GATE_SL[gate]
            sg = 2.0 if gate == "g" else 1.0
            m = np.zeros((Kq, 128), np.float32)
            for j in range(nl):
                l = l0 + j
                m[25 * j:25 * j + 25, 25 * j:25 * j + 25] = \
                    (sg * W_hh[l][gs]).T
                if j >= 1:
                    m[25 * (j - 1):25 * j, 25 * j:25 * j + 25] = \
                        (sg * W_ih[l][gs]).T
                m[Kq - 1, 25 * j:25 * j + 25] = sg * b[l][gs]
            m[25 * nl:25 * nl + 25, 0:25] = (sg * W_ih[l0][gs]).T
            add((q, gate), m)

    for g in range(4):
        m = np.zeros((26, 103), np.float32)
        m[0:25, 32 * g:32 * g + 7] = fc_w.T
        m[25, 32 * g:32 * g + 7] = fc_b
        add(("fc", g), m)
    m = np.zeros((103, 4), np.float32)
    for g in range(4):
        m[32 * g:32 * g + 7, g] = 1.0
    add(("sel",), m)

    ncols = sum(x.shape[1] for x in mats)
    blob = np.zeros((128, ncols), np.float16)
    c = 0
    for m in mats:
        blob[:m.shape[0], c:c + m.shape[1]] = m.astype(np.float16)
        c += m.shape[1]
    return blob, cols


_WFCTR = [0]


def _split_excess_waits(nc):
    """Walrus allows 1 sync-wait per instruction (2 for EventSemaphore).
    Hoist extras onto preceding same-engine NOPs."""
    for fn in nc.m.functions:
        for blk in fn.blocks:
            insts = list(blk.instructions)
            out = []
            changed = False
            for inst in insts:
                si = inst.sync_info
                cap = 2 if type(inst).__name__ == "InstEventSemaphore" else 1
                if si is not None and si.on_wait is not None and len(si.on_wait) > cap:
                    waits = list(si.on_wait)
                    extra, keep = waits[:-cap], waits[-cap:]
                    for wt in extra:
                        _WFCTR[0] += 1
                        out.append(mybir.InstNoOp(
                            name=f"I-waitfix-{_WFCTR[0]}", opcode="NoOp",
                            engine=inst.engine, ins=[], outs=[],
                            sync_info=mybir.SyncInfo(on_wait=[wt], on_update=[]),
                        ))
                    inst.sync_info = mybir.SyncInfo(
                        on_wait=keep, on_update=list(si.on_update))
                    changed = True
                out.append(inst)
            if changed:
                blk.instructions = out


# ---------------------------------------------------------------- program
_PROG_CACHE = {}


def _build_program(TL, wcols, nwcol, patch_waits=True):
    nc = bass.Bass()
    NW = TL + PIPE
    NCH = TL * BS // 512
    NGRP = NCH // 4

    xT = nc.declare_dram_parameter("xT", [25, (TL + 16) * BS], F16, isOutput=False)
    wpack = nc.declare_dram_parameter("wpack", [128, nwcol], F16, isOutput=False)
    hpre = nc.declare_dram_parameter("hpre", [100, 3 * BS], F16, isOutput=False)
    cpre = nc.declare_dram_parameter("cpre", [100, 3 * BS], F16, isOutput=False)
    onesd = nc.declare_dram_parameter("onesd", [1, 512], F16, isOutput=False)
    exp_out = nc.declare_dram_parameter("exp_out", [NGRP, 4, 7, 512], F16, isOutput=True)
    sum_out = nc.declare_dram_parameter("sum_out", [NGRP, 4, 512], F32, isOutput=True)
    h9d = nc.dram_tensor("h9d", [25, TL * BS], F16)

    GQ = {"i": 0, "f": 1, "g": 2, "o": 3}   # psum quarter per gate

    def first_w(q):
        return QUADS[q][0] + SKEW[q]

    with TileContext(nc) as tc:
      with tc.tile_pool(name="glob", bufs=1) as glob:
        wsb = glob.tile([128, nwcol], F16)
        with (
            tc.tile_pool(name="pers", bufs=1) as pers,
            tc.tile_pool(name="wps", bufs=1, space="PSUM") as pps,
        ):
            Ht = [[pers.tile([128, BS], F16, name=f"H{q}p{p}") for p in range(2)]
                  for q in range(3)]
            Ct = [pers.tile([128, 3 * BS], F16, name=f"C{p}") for p in range(2)]
            SG = [[pers.tile([128, 4 * BS], F16, name=f"SG{q}p{p}") for p in range(2)]
                  for q in range(3)]
            SC = [pers.tile([128, 3 * BS], F16, name=f"SC{p}") for p in range(2)]
            Ut = [pers.tile([128, BS], F16, name=f"U{q}") for q in range(3)]
            Vt = [pers.tile([128, BS], F16, name=f"V{q}") for q in range(3)]
            Wt = [pers.tile([128, BS], F16, name=f"W{q}") for q in range(3)]
            pG = [pps.tile([128, 4 * BS], F32, name=f"pG{q}") for q in range(3)]

            # ---- init
            nc.sync.dma_start(out=wsb[:, :], in_=wpack[:, :])
            for q, (l0, nl) in enumerate(QUADS):
                R = 25 * nl
                for p in range(2):
                    nc.sync.dma_start(
                        out=Ht[q][p][0:R, :],
                        in_=hpre[0:R, q * BS:(q + 1) * BS])
                    nc.sync.dma_start(
                        out=Ht[q][p][R + 25:R + 26, :], in_=onesd[:, 0:BS])
            for p in range(2):
                nc.sync.dma_start(out=Ct[p][0:100, :], in_=cpre[:, :])
            # x(0) -> H0 parity1 input slot, x(1) -> parity0
            nc.sync.dma_start(out=Ht[0][1][100:125, :], in_=xT[:, 0:BS])
            nc.sync.dma_start(out=Ht[0][0][100:125, :], in_=xT[:, BS:2 * BS])

            # ---- wavefront
            for w in range(NW):
                p, pp = w % 2, (w + 1) % 2
                act = [q for q in range(3) if w >= first_w(q)]
                rws = {q: 25 * min(QUADS[q][1], w - first_w(q) + 1) for q in act}
                # matmuls (all quads first)
                for q in act:
                    l0, nl = QUADS[q]
                    Kq = 25 * nl + 26
                    for gate in "ifgo":
                        co = GQ[gate] * BS
                        wc = wcols[(q, gate)]
                        nc.tensor.matmul(
                            pG[q][0:128, co:co + BS],
                            wsb[0:Kq, wc:wc + 128],
                            Ht[q][pp][0:Kq, :],
                            start=True, stop=True,
                        )
                # one sigmoid over all four gate quarters
                for q in act:
                    R = 25 * QUADS[q][1]
                    nc.scalar.activation(
                        SG[q][p][0:R, :], pG[q][0:R, :], AF.Sigmoid)
                # DVE cell math
                for q in act:
                    R = 25 * QUADS[q][1]
                    rw = rws[q]
                    sg = SG[q][p]
                    # w = 2*sig(2g) - 1 = tanh(g)
                    nc.vector.tensor_scalar(
                        Wt[q][0:R, :], sg[0:R, 2 * BS:3 * BS], 2.0, -1.0,
                        ALU.mult, ALU.add)
                    nc.vector.tensor_mul(
                        Ut[q][0:R, :], Wt[q][0:R, :], sg[0:R, 0:BS])
                    nc.vector.tensor_mul(
                        Vt[q][0:R, :], sg[0:R, BS:2 * BS],
                        Ct[pp][0:R, q * BS:q * BS + BS])
                    nc.vector.tensor_add(
                        Ct[p][0:rw, q * BS:q * BS + BS],
                        Ut[q][0:rw, :], Vt[q][0:rw, :])
                # z = tanh(c') per quad
                for q in act:
                    R = 25 * QUADS[q][1]
                    nc.scalar.activation(
                        SC[p][0:R, q * BS:q * BS + BS],
                        Ct[p][0:R, q * BS:q * BS + BS], AF.Tanh)
                # h = z * so
                for q in act:
                    rw = rws[q]
                    nc.vector.tensor_mul(
                        Ht[q][p][0:rw, :],
                        SC[p][0:rw, q * BS:q * BS + BS],
                        SG[q][p][0:rw, 3 * BS:4 * BS])
                # cross-quad input-slot hand-off (consumed at w+2)
                if w >= 3:
                    nc.gpsimd.dma_start(
                        out=Ht[1][pp][100:125, :], in_=Ht[0][p][75:100, :])
                if w >= 8:
                    nc.gpsimd.dma_start(
                        out=Ht[2][pp][50:75, :], in_=Ht[1][p][75:100, :])
                # x prefetch for window w+2
                nc.sync.dma_start(
                    out=Ht[0][pp][100:125, :],
                    in_=xT[:, (w + 2) * BS:(w + 3) * BS])
                # h9 export (layer 9 = q2 slot 1, lands at w = t + 11)
                if w >= PIPE:
                    nc.sync.dma_start(
                        out=h9d[:, (w - PIPE) * BS:(w - PIPE + 1) * BS],
                        in_=Ht[2][p][25:50, :])

        # ---------------- tail: FC + exp + band sums
        with (
            tc.tile_pool(name="tailw", bufs=4) as twp,
            tc.tile_pool(name="tps", bufs=2, space="PSUM") as tps,
            tc.tile_pool(name="tpers", bufs=1) as tpers,
        ):
            rhs = [tpers.tile([26, 512], F16, name=f"rhs{i}") for i in range(2)]
            for i in range(2):
                nc.sync.dma_start(out=rhs[i][25:26, :], in_=onesd[:, :])
            for j in range(NGRP):
                fcps = tps.tile([128, 512], F32, tag="fcps")
                for g in range(4):
                    ch = 4 * j + g
                    rt = rhs[ch % 2]
                    nc.sync.dma_start(
                        out=rt[0:25, :], in_=h9d[:, ch * 512:(ch + 1) * 512])
                    nc.tensor.matmul(
                        fcps[0:103, :],
                        wsb[0:26, wcols[("fc", g)]:wcols[("fc", g)] + 103],
                        rt[0:26, :],
                        start=(g == 0), stop=(g == 3),
                    )
                esb = twp.tile([128, 512], F16, tag="esb")
                nc.scalar.activation(esb[0:103, :], fcps[0:103, :], AF.Exp)
                sps = tps.tile([4, 512], F32, tag="sps")
                nc.tensor.matmul(
                    sps[0:4, :],
                    wsb[0:103, wcols[("sel",)]:wcols[("sel",)] + 4],
                    esb[0:103, :],
                    start=True, stop=True,
                )
                ssb = twp.tile([32, 512], F32, tag="ssb")
                nc.scalar.copy(ssb[0:4, :], sps[0:4, :])
                for g in range(4):
                    nc.gpsimd.dma_start(
                        out=exp_out[j, g, :, :], in_=esb[32 * g:32 * g + 7, :])
                nc.sync.dma_start(out=sum_out[j, :, :], in_=ssb[0:4, :])

    if patch_waits:
        _split_excess_waits(nc)
    return nc


def _get_program(TL, wcols, nwcol):
    if TL not in _PROG_CACHE:
        _PROG_CACHE[TL] = _build_program(TL, wcols, nwcol)
    return _PROG_CACHE[TL]


# ---------------------------------------------------------------- kernel
def kernel(x, h0, c0, W_ih, W_hh, b, fc_w, fc_b, _trace=False, _TL=None):
    x = np.asarray(x, np.float32)
    h0 = np.asarray(h0, np.float32)
    c0 = np.asarray(c0, np.float32)
    TL = (T // NT + WARM) if _TL is None else _TL
    TCH = TL - WARM                 # real timesteps per time-shard
    Teff = NT * TCH

    blob, wcols = _pack_weights(
        np.asarray(W_ih, np.float32), np.asarray(W_hh, np.float32),
        np.asarray(b, np.float32), np.asarray(fc_w, np.float32),
        np.asarray(fc_b, np.float32))

    nc = _get_program(TL, wcols, blob.shape[1])

    in_maps = []
    for tb in range(NT):
        t0 = 0 if tb == 0 else tb * TCH - WARM
        for bb in range(NBS):
            bsl = slice(bb * BS, (bb + 1) * BS)
            xt = np.zeros((25, (TL + 16) * BS), np.float16)
            xc = x[t0:t0 + TL, bsl, :]              # [TL, BS, 25]
            xt[:, 0:TL * BS] = xc.transpose(2, 0, 1).reshape(25, -1)
            hp = np.zeros((100, 3 * BS), np.float16)
            cp = np.zeros((100, 3 * BS), np.float16)
            if tb == 0:
                for q, (l0, nl) in enumerate(QUADS):
                    for j in range(nl):
                        hp[25 * j:25 * j + 25, q * BS:(q + 1) * BS] = \
                            h0[l0 + j, bsl, :].T.astype(np.float16)
                        cp[25 * j:25 * j + 25, q * BS:(q + 1) * BS] = \
                            c0[l0 + j, bsl, :].T.astype(np.float16)
            in_maps.append({"xT": xt, "wpack": blob, "hpre": hp, "cpre": cp,
                            "onesd": np.ones((1, 512), np.float16)})

    res = run_bass_kernel_spmd(nc, in_maps, list(range(NC)), trace=_trace)

    # host: softmax divide + assemble
    y = np.empty((Teff, B, NCLS), np.float32)
    ci = 0
    for tb in range(NT):
        for bb in range(NBS):
            e = res.results[ci]["exp_out"].astype(np.float32)  # [NGRP,4,7,512]
            s = res.results[ci]["sum_out"]                     # [NGRP,4,512]
            yc = (e / s[:, :, None, :]).transpose(0, 1, 3, 2)  # [NGRP,4,512,7]
            yc = yc.reshape(TL, BS, NCLS)
            tsl = yc[0:TCH] if tb == 0 else yc[WARM:WARM + TCH]
            y[tb * TCH:(tb + 1) * TCH, bb * BS:(bb + 1) * BS, :] = tsl
            ci += 1
    out = y.reshape(Teff * B, NCLS)
    return (out, res) if _trace else out


if __name__ == "__main__":
    pass
